# revision 2
# baseline (speedup 1.0000x reference)
"""Curvature stencil (TV-flow) kernel for Trainium2.

Extended-grid formulation: host pre-pads each 1024x1024 image to 1026x1026
with reflect rows/cols chosen so that one uniform stencil computes all
boundary cases exactly:
  ue[0]=u[1], ue[1..1024]=u[0..1023], ue[1025]=u[1022]; cols likewise.
On the 1025x1025 "P-grid" (r,c = 0..1024):
  dxf[r,c] = ue[r+1,c]-ue[r,c];  dyf[r,c] = ue[r,c+1]-ue[r,c]
  F = sqrt(dxf^2+dyf^2+eps);  P = dxf/F;  Q = dyf/F
  out[i,j] = P[i+1,j+1]-P[i,j+1] + Q[i+1,j+1]-Q[i+1,j]

Layout: one SBUF tile per image; partition p holds P-grid rows 8p..8p+8
in its free dim (9 overlapping row-blocks; +1 halo row for dxf), so every
shift is a free-dim AP offset - zero cross-partition traffic.

Two custom fused DVE ops (registered at import):
  CURV_SQSUM:    out = in0^2 + in1^2
  CURV_RECIPMUL: out = recip1(in0) * in1   (bit-trick seed + 1 Newton step,
                                            max rel err ~0.17%)
11 instructions per image total.
"""

import sys

sys.path.insert(0, "/opt/trn_rl_repo")

import numpy as np

import concourse.bass as bass
import concourse.tile as tile
import concourse.dve_ops as dve_ops
from concourse import bacc, mybir
from concourse.dve_spec import Spec, Src0, Src1, C0, C1, Bin, AluOp
from concourse.dve_ops import DveOp
from contextlib import ExitStack

EPS = 1e-16
RC0, RC1 = -0.2354979, 2.00173242  # minimax pair for 1-NR reciprocal
B, H, W = 16, 1024, 1024
HE, WE = H + 2, W + 2      # extended grid
K = 8                      # output rows per partition
P_ = 128
NB = K + 1                 # P-grid row-blocks per partition (overlap 1)
NU = K + 2                 # ue row-blocks per partition
NCORES = 4                 # cores actually used (8 available; >4 contend)
M = B // NCORES            # images per core
DT = mybir.dt.float32

_CACHE = {}


def _register_ops():
    if "CURV_SQSUM" in dve_ops._SUB_OPCODE_FOR_NAME:
        return
    sqsum = DveOp(
        "CURV_SQSUM",
        Spec(
            body=Src0 * Src0 + Src1 * Src1,
            reference=lambda in0, in1, c0, c1, c2: in0 * in0 + in1 * in1,
        ),
        subdim=False,
        uops_sha={"v3": "cd4bd6e1c27efd14", "v4": "121e32d8332f5047"},
    )
    _not = Bin(AluOp.BITWISE_NOT, Src0, Src0)
    _y0 = _not * C0
    _y1 = _y0 * (C1 - Src0 * _y0)

    def _ref_recipmul(in0, in1, c0, c1, c2):
        nx = (~in0.view(np.int32)).view(np.float32)
        y0 = nx * c0
        return (y0 * (c1 - in0 * y0)) * in1

    recipmul = DveOp(
        "CURV_RECIPMUL",
        Spec(body=_y1 * Src1, reference=_ref_recipmul),
        subdim=False,
        uops_sha={"v3": "e11870b101db7dce", "v4": "0eb0cb68104d73b5"},
    )
    dve_ops.OPS.append(sqsum)
    dve_ops.OPS.append(recipmul)
    dve_ops._SUB_OPCODE_FOR_NAME = {
        op.name: dve_ops._CUSTOM_DVE_ROW_BASE + i for i, op in enumerate(dve_ops.OPS)
    }


_register_ops()
SQSUM = dve_ops.OPS[-2]
RECIPMUL = dve_ops.OPS[-1]


def _dram_ap(t, dims, offset):
    ap = t[:].copy()
    ap.ap = type(ap.ap)(dims)
    ap.offset = offset
    return ap


def _build(repeat=1, ncores=NCORES, hw_loop=False):
    m_per = B // ncores
    nc = bacc.Bacc("TRN2", target_bir_lowering=False, debug=False)
    u_ext = nc.declare_dram_parameter("u", [m_per * HE, WE], DT, isOutput=False)
    out_ext = nc.declare_dram_parameter("out", [m_per * H, W], DT, isOutput=True)

    NF = NB * WE           # 9*1026 free elems per partition for P-grid bufs
    NV = NF - 1            # valid flat length (last col of last block unused)
    with tile.TileContext(nc) as tc, ExitStack() as ctx:
        pool = ctx.enter_context(tc.tile_pool(name="p", bufs=1))
        cpool = ctx.enter_context(tc.tile_pool(name="c", bufs=1))
        eps_t = cpool.tile([P_, 1], DT, tag="eps")
        nc.vector.memset(eps_t[:], EPS)
        if hw_loop:
            rep_ctx = tc.For_i(0, repeat)
            rep_range = [0]
        else:
            rep_ctx = None
            rep_range = range(repeat)
        with rep_ctx if rep_ctx is not None else ExitStack():
            for _rep in rep_range:
                _emit_images(nc, pool, eps_t, u_ext, out_ext, m_per)

    nc.finalize()
    return nc


def _emit_images(nc, pool, eps_t, u_ext, out_ext, m_per):
    NF = NB * WE
    NV = NF - 1
    if True:
        if True:
            for m in range(m_per):
                u3 = pool.tile([P_, NU * WE], DT, tag="A")
                src = _dram_ap(
                    u_ext, [(K * WE, P_), (WE, NU), (1, WE)], m * HE * WE
                )
                nc.sync.dma_start(
                    u3[:].rearrange("p (b j) -> p b j", b=NU, j=WE), src
                )

                # dxf[b,j] = ue[b+1,j] - ue[b,j], flat over 9 blocks
                dxf = pool.tile([P_, NF], DT, tag="b1")
                nc.vector.tensor_sub(dxf[:], u3[:, WE : NU * WE], u3[:, 0:NF])

                # dyf[b,j] = ue[b,j+1] - ue[b,j], flat (garbage at j=WE-1)
                dyf = pool.tile([P_, NF], DT, tag="b2")
                nc.gpsimd.tensor_sub(
                    dyf[:, 0:NV], u3[:, 1 : NF + 1 - 1], u3[:, 0:NV]
                )

                f2 = pool.tile([P_, NF], DT, tag="b3")
                nc.vector._custom_dve(
                    SQSUM, out=f2[:, 0:NV], in0=dxf[:, 0:NV], in1=dyf[:, 0:NV]
                )

                ff = pool.tile([P_, NF], DT, tag="b4")
                nc.scalar.activation(
                    ff[:, 0:NV],
                    f2[:, 0:NV],
                    mybir.ActivationFunctionType.Sqrt,
                    bias=eps_t[:],
                )

                pt = pool.tile([P_, NF], DT, tag="b3")
                nc.vector._custom_dve(
                    RECIPMUL,
                    out=pt[:, 0:NV],
                    in0=ff[:, 0:NV],
                    in1=dxf[:, 0:NV],
                    s0=RC0,
                    s1=RC1,
                )
                qt = pool.tile([P_, NF], DT, tag="A")
                nc.vector._custom_dve(
                    RECIPMUL,
                    out=qt[:, 0:NV],
                    in0=ff[:, 0:NV],
                    in1=dyf[:, 0:NV],
                    s0=RC0,
                    s1=RC1,
                )

                pv = pt[:].rearrange("p (b j) -> p b j", b=NB, j=WE)
                qv = qt[:].rearrange("p (b j) -> p b j", b=NB, j=WE)

                # t2[b,j] = P[b+1, j+1] - P[b, j+1]
                t2 = pool.tile([P_, K * W], DT, tag="b4")
                t2v = t2[:].rearrange("p (b j) -> p b j", b=K, j=W)
                nc.gpsimd.tensor_sub(
                    t2v[:, :, :], pv[:, 1:NB, 1 : W + 1], pv[:, 0:K, 1 : W + 1]
                )
                # t1[b,j] = Q[b+1, j+1] - Q[b+1, j]
                t1 = pool.tile([P_, K * W], DT, tag="b1")
                t1v = t1[:].rearrange("p (b j) -> p b j", b=K, j=W)
                nc.gpsimd.tensor_sub(
                    t1v[:, :, :], qv[:, 1:NB, 1 : W + 1], qv[:, 1:NB, 0:W]
                )
                ot = pool.tile([P_, K * W], DT, tag="b2")
                nc.vector.tensor_add(ot[:], t2[:], t1[:])

                dst = _dram_ap(out_ext, [(K * W, P_), (1, K * W)], m * H * W)
                nc.sync.dma_start(dst, ot[:])


def _prep_core(x):
    """x: [m, H, W] f32 -> ue [m*HE, WE] extended grid."""
    m = x.shape[0]
    ue = np.empty((m, HE, WE), dtype=np.float32)
    ue[:, 1 : H + 1, 1 : W + 1] = x
    ue[:, 0, 1 : W + 1] = x[:, 1, :]
    ue[:, H + 1, 1 : W + 1] = x[:, H - 2, :]
    ue[:, :, 0] = ue[:, :, 2]
    ue[:, :, W + 1] = ue[:, :, W - 1]
    return ue.reshape(m * HE, WE)


def kernel(u):
    from concourse.bass_utils import run_bass_kernel_spmd

    x = np.asarray(u, dtype=np.float32).reshape(B, H, W)
    if "nc" not in _CACHE:
        _CACHE["nc"] = _build()
    nc = _CACHE["nc"]

    in_maps = []
    for c in range(NCORES):
        in_maps.append({"u": _prep_core(x[c * M : (c + 1) * M])})

    res = run_bass_kernel_spmd(nc, in_maps, core_ids=list(range(NCORES)))
    out = np.stack([r["out"] for r in res.results])  # [ncores, M*H, W]
    return out.reshape(B, H, W, 1)
